# revision 15
# baseline (speedup 1.0000x reference)
"""DisparityWarp Trainium2 kernel (Bass/Tile).

Contract: kernel(src, disparity) takes FULL inputs
  src [8, 32, 384, 768] f32, disparity [8, 1, 384, 768] f32
and returns the FULL output [8, 32, 384, 768] f32 of
  grid_sample(src, grid, bilinear, zeros, align_corners=False)
with grid_x = 2*(xx - disp)/(W-1) - 1, grid_y = 2*yy/(H-1) - 1.

Sharding: pure data parallel, one batch per NeuronCore (8 cores).

Per-core algorithm (all compute on device from src+disparity; host only
prepares data-INDEPENDENT geometry constants):
  unnormalized coords: ix = (x - d)*W/(W-1) - 0.5, iy = y*H/(H-1) - 0.5.
  The vertical lerp has per-row constant weights; the horizontal warp is a
  banded linear map out[c,x] = sum_x' vrow[c,x'] * hat(ix - x') with
  hat(u) = max(0, 1-|u|).

  - blocks: output cols [94j, 94j+94); x' window [94j-33, 94j+95) (128 wide).
  - D[p, x] = ixm1[x] - (p+1) via a K=3 fp16 matmul per row
    (rhs rows: ones / round(ixm1) / frac -- fp16-exact except frac ~5e-4).
  - A = Abs(D) on ScalarE; Wneg = min(A-1, 0) = -hat on VectorE (fp16).
  - vertical blend fused into the PE transpose: stationary = 4 src rows
    (128 partitions = 4r x 32c), streamed = constant 128x96 blend matrix
    with NEGATED coefs (sign cancels Wneg) -> VT[x'loc, (3y, 32c)].
  - gather matmuls: out[32i:, 94j:] += VT_j[:, 32i:+32].T @ Wneg[:, 94j:+94]
    (3 rows stacked on PSUM partitions via column tiling).
"""

import sys

if "/opt/trn_rl_repo" not in sys.path:
    sys.path.insert(0, "/opt/trn_rl_repo")

from contextlib import ExitStack

import numpy as np

import concourse.bass as bass
import concourse.mybir as mybir
from concourse import bacc
from concourse.tile import TileContext

F32 = mybir.dt.float32
F16 = mybir.dt.float16
I32 = mybir.dt.int32
AF = mybir.ActivationFunctionType
ALU = mybir.AluOpType

B, C, H, W = 8, 32, 384, 768
S = W / (W - 1)
BLK = 94           # output columns per block
NB = 9             # ceil(W / BLK)
WIN = 128          # x' window width per block
PADL = 33          # left zero pad of S4 (x' = col - PADL)
S4W = BLK * (NB - 1) + WIN  # 880; right pad = 79
GRP = 3            # output rows per group
NG = H // GRP      # 128 groups
N_CORES = 8


# ---------------------------------------------------------------- constants
def _vert_coefs():
    yy = np.arange(H, dtype=np.float64)
    iy = yy * (H / (H - 1)) - 0.5
    y0 = np.floor(iy).astype(np.int64)
    fy = iy - y0
    a = (1.0 - fy) * ((y0 >= 0) & (y0 < H))
    b = fy * ((y0 + 1 >= 0) & (y0 + 1 < H))
    return a, b, y0


def _host_constants():
    a, b, y0 = _vert_coefs()
    # V2NEG [128 p=(4r,32c), NG, 96 m=(3i,32c)]  fp16, negated blend coefs
    v2 = np.zeros((4, C, NG, GRP, C), dtype=np.float32)
    quad_bases = []
    for g in range(NG):
        ys = [GRP * g + i for i in range(GRP)]
        qbase = min(max(int(y0[ys[0]]), 0), H - 4)
        quad_bases.append(qbase)
        for i, y in enumerate(ys):
            ra = int(y0[y]) - qbase
            rb = ra + 1
            for c in range(C):
                if a[y] != 0.0:
                    assert 0 <= ra <= 3
                    v2[ra, c, g, i, c] += -a[y]
                if b[y] != 0.0:
                    assert 0 <= rb <= 3
                    v2[rb, c, g, i, c] += -b[y]
    v2neg = v2.reshape(4 * C, NG, GRP * C).astype(np.float16)

    ld3 = np.stack([
        -(np.arange(WIN, dtype=np.float32) + 1.0),
        np.ones(WIN, dtype=np.float32),
        np.ones(WIN, dtype=np.float32),
    ]).astype(np.float16)                                   # [3, 128]

    x = np.arange(W, dtype=np.float64)
    base = BLK * (np.arange(W) // BLK) - PADL
    cf = (x * S - 0.5 - base + 1.0).astype(np.float32)[None, :]   # [1, W]
    return v2neg, ld3, cf, quad_bases


# ---------------------------------------------------------------- program
def build_nc(ngroups=NG):
    _, _, _, quad_bases = _host_constants()
    nc = bacc.Bacc("TRN2", target_bir_lowering=False, debug=False)

    src = nc.dram_tensor("src", [C, H, W], F32, kind="ExternalInput").ap()
    disp = nc.dram_tensor("disp", [H, W], F32, kind="ExternalInput").ap()
    v2d = nc.dram_tensor("v2neg", [4 * C, NG, GRP * C], F16,
                         kind="ExternalInput").ap()
    ld3d = nc.dram_tensor("ld3", [3, WIN], F16, kind="ExternalInput").ap()
    cfd = nc.dram_tensor("cf", [1, W], F32, kind="ExternalInput").ap()
    outd = nc.dram_tensor("out", [C, H, W], F32, kind="ExternalOutput").ap()

    ngr = min(ngroups, NG)

    with ExitStack() as ctx:
        tc = ctx.enter_context(TileContext(nc))
        singles = ctx.enter_context(tc.tile_pool(name="singles", bufs=1))
        ph1 = ctx.enter_context(tc.tile_pool(name="ph1", bufs=2))
        vtsbp = ctx.enter_context(tc.tile_pool(name="vtsbp", bufs=2))
        ap_ = ctx.enter_context(tc.tile_pool(name="ap", bufs=2))
        wp = ctx.enter_context(tc.tile_pool(name="wp", bufs=2))
        outsbp = ctx.enter_context(tc.tile_pool(name="outsbp", bufs=2))
        vtpp = ctx.enter_context(tc.tile_pool(name="vtpp", bufs=1, space="PSUM"))
        dpp = ctx.enter_context(tc.tile_pool(name="dpp", bufs=2, space="PSUM"))
        outpp = ctx.enter_context(tc.tile_pool(name="outpp", bufs=1, space="PSUM"))

        # ---- constants ----
        v2sb = singles.tile([4 * C, NG, GRP * C], F16)
        nc.sync.dma_start(out=v2sb, in_=v2d)
        ld3sb = singles.tile([3, WIN], F16)
        nc.sync.dma_start(out=ld3sb, in_=ld3d)
        cfb = singles.tile([128, W], F32)
        nc.sync.dma_start(out=cfb, in_=cfd.to_broadcast((128, W)))

        # ---- persistent rings (pads / ones written once) ----
        NRING = 3
        s4ring = [singles.tile([128, S4W], F16, tag=f"s4r{k}", name=f"s4r{k}")
                  for k in range(NRING)]
        for t_ in s4ring:
            nc.gpsimd.memset(t_[:, 0:PADL], 0.0)
            nc.gpsimd.memset(t_[:, PADL + W:S4W], 0.0)
        slabring = [singles.tile([3, 2 * GRP, W], F16, tag=f"slabr{k}",
                                 name=f"slabr{k}")
                    for k in range(NRING)]
        for t_ in slabring:
            nc.gpsimd.memset(t_[0:1, :, :], 1.0)

        # Absorber matmuls: LDWEIGHTS supports only ONE sync wait, so have
        # the PE observe multi-producer init writes via dummy Matmults
        # (whose wait budget is larger) before any real LDW touches them.
        dummyp = vtpp.tile([1, 2 * NRING], F32, tag="vtp", name="dummyp")
        for k in range(NRING):
            nc.tensor.matmul(dummyp[0:1, 2 * k:2 * k + 1],
                             ld3sb[0:1, 0:1], s4ring[k][0:1, 0:1],
                             start=True, stop=True)
            nc.tensor.matmul(dummyp[0:1, 2 * k + 1:2 * k + 2],
                             ld3sb[0:1, 0:1], slabring[k][0:1, 0:1, 0:1],
                             start=True, stop=True)

        # ---- phase 1: ixm1 -> int/frac fp16 tiles [128, 3, W] (row-major) --
        int16 = singles.tile([128, 3, W], F16)
        frac16 = singles.tile([128, 3, W], F16)
        nrows = GRP * ngr
        nt = (nrows + 127) // 128
        for t in range(nt):
            r0 = 128 * t
            nr = min(128, H - r0)
            dt_ = ph1.tile([128, W], F32)
            nc.sync.dma_start(out=dt_[:nr], in_=disp[r0:r0 + nr, :])
            ixm1 = ph1.tile([128, W], F32)
            nc.vector.tensor_scalar_mul(ixm1[:nr], dt_[:nr], -float(S))
            nc.vector.tensor_add(ixm1[:nr], ixm1[:nr], cfb[:nr])
            iv = ph1.tile([128, W], I32)
            nc.vector.tensor_copy(iv[:nr], ixm1[:nr])
            fv = ph1.tile([128, W], F32)
            nc.vector.tensor_copy(fv[:nr], iv[:nr])
            nc.vector.tensor_copy(int16[:nr, t, :], fv[:nr])
            fr = ph1.tile([128, W], F32)
            nc.vector.tensor_sub(fr[:nr], ixm1[:nr], fv[:nr])
            nc.vector.tensor_copy(frac16[:nr, t, :], fr[:nr])

        # ---- phase 2: main loop (pairs of 3-row groups) ----
        def stage_rows(dst, slot0, src_t, y_lo, cnt):
            # dst[0:1, slot0:slot0+cnt, :] <- src_t rows y_lo..y_lo+cnt
            # (src partition = y % 128, free slot = y // 128)
            done = 0
            while done < cnt:
                y = y_lo + done
                p, t = y % 128, y // 128
                n = min(cnt - done, 128 - p)
                nc.sync.dma_start(
                    out=dst[0:1, slot0 + done:slot0 + done + n, :],
                    in_=src_t[p:p + n, t, :])
                done += n

        for gp in range(0, ngr, 2):
            npair = min(2, ngr - gp)
            slab = slabring[(gp // 2) % NRING]
            stage_rows(slab[1:2], 0, int16, GRP * gp, GRP * npair)
            stage_rows(slab[2:3], 0, frac16, GRP * gp, GRP * npair)
            # absorber: PE observes the staging DMAs here, not on an LDW
            dmy = vtpp.tile([1, 1], F32, tag="vtp", name="dmy")
            nc.tensor.matmul(dmy, ld3sb[0:3, 0:1], slab[0:3, 0:1, 0:1],
                             start=True, stop=True)

            for gi in range(npair):
                g = gp + gi
                qbase = quad_bases[g]
                s4 = s4ring[g % NRING]
                in_ap = src[:, qbase:qbase + 4, :].rearrange("c r x -> r c x")
                nc.gpsimd.dma_start(out=s4[:, PADL:PADL + W], in_=in_ap)

                # blend-transpose: 9 windows
                vtp = vtpp.tile([128, 1024], F32)
                for j in range(NB):
                    pc = 96 * j if j < 5 else 512 + 96 * (j - 5)
                    nc.tensor.matmul(
                        vtp[:, pc:pc + GRP * C],
                        s4[:, BLK * j:BLK * j + WIN],
                        v2sb[:, g, :],
                        start=True, stop=True,
                    )
                vtsb = vtsbp.tile([128, NB * GRP * C], F16)
                nc.scalar.copy(vtsb[:, 0:480], vtp[:, 0:480])
                nc.scalar.copy(vtsb[:, 480:864], vtp[:, 512:896])

                outp = outpp.tile([96, 1024], F32)
                for i in range(GRP):
                    dp = dpp.tile([128, 1024], F32)
                    rhs = slab[0:3, GRP * gi + i, :]
                    nc.tensor.matmul(dp[:, 0:512], ld3sb[:], rhs[:, 0:512],
                                     start=True, stop=True)
                    nc.tensor.matmul(dp[:, 512:768], ld3sb[:], rhs[:, 512:W],
                                     start=True, stop=True)
                    asb = ap_.tile([128, W], F16)
                    nc.scalar.activation(asb, dp[:, 0:W], AF.Abs)
                    wsb = wp.tile([128, W], F16)
                    nc.vector.tensor_scalar(out=wsb, in0=asb, scalar1=1.0,
                                            scalar2=0.0, op0=ALU.subtract,
                                            op1=ALU.min)
                    for j in range(NB):
                        n = min(BLK, W - BLK * j)
                        oc = BLK * j if j < 5 else 512 + BLK * (j - 5)
                        nc.tensor.matmul(
                            outp[32 * i:32 * i + 32, oc:oc + n],
                            vtsb[:, 96 * j + 32 * i:96 * j + 32 * i + 32],
                            wsb[:, BLK * j:BLK * j + n],
                            start=True, stop=True,
                            tile_position=(0, 32 * i),
                        )
                outsb = outsbp.tile([96, W], F32)
                nc.scalar.copy(outsb[:, 0:470], outp[:, 0:470])
                nc.vector.tensor_copy(outsb[:, 470:W], outp[:, 512:810])
                out_ap = outd[:, GRP * g:GRP * g + GRP, :].rearrange(
                    "c r x -> r c x")
                nc.sync.dma_start(out=out_ap, in_=outsb)

    nc.finalize()
    return nc


_NC_CACHE = {}


def _get_nc(ngroups=NG):
    if ngroups not in _NC_CACHE:
        _NC_CACHE[ngroups] = build_nc(ngroups)
    return _NC_CACHE[ngroups]


# ---------------------------------------------------------------- entry
def kernel(src: np.ndarray, disparity: np.ndarray) -> np.ndarray:
    from concourse.bass_utils import run_bass_kernel_spmd

    src = np.ascontiguousarray(np.asarray(src), dtype=np.float32)
    disparity = np.ascontiguousarray(np.asarray(disparity), dtype=np.float32)
    v2neg, ld3, cf, _ = _host_constants()
    nc = _get_nc()
    in_maps = []
    for b in range(B):
        in_maps.append({
            "src": src[b],
            "disp": disparity[b, 0],
            "v2neg": v2neg,
            "ld3": ld3,
            "cf": cf,
        })
    res = run_bass_kernel_spmd(nc, in_maps, core_ids=list(range(N_CORES)))
    out = np.stack([res.results[b]["out"] for b in range(B)])
    return out.astype(np.float32)


# ---------------------------------------------------------------- sim test
def _sim_check(ngroups=2):
    from concourse.bass_interp import CoreSim

    rng = np.random.default_rng(0)
    src = rng.standard_normal((C, H, W)).astype(np.float32)
    disp = (rng.random((H, W)) * 32.0).astype(np.float32)
    v2neg, ld3, cf, _ = _host_constants()

    nc = build_nc(ngroups)
    sim = CoreSim(nc)
    for name, val in (("src", src), ("disp", disp), ("v2neg", v2neg),
                      ("ld3", ld3), ("cf", cf)):
        sim.tensor(name)[:] = val
    sim.simulate(check_with_hw=False)
    got = np.array(sim.tensor("out"))

    from proto import reference
    ref = reference(src[None], disp[None, None])[0]
    ys = slice(0, GRP * ngroups)
    diff = got[:, ys] - ref[:, ys]
    rel = np.linalg.norm(diff) / np.linalg.norm(ref[:, ys])
    print(f"sim rows[0:{GRP * ngroups}]  max abs "
          f"{np.abs(diff).max():.3e}  rel l2 {rel:.3e}")
    return rel


if __name__ == "__main__":
    ng = int(sys.argv[1]) if len(sys.argv) > 1 else 2
    _sim_check(ng)
